# revision 30
# baseline (speedup 1.0000x reference)
"""TRN2 Bass kernel for nn_Attention_86260123173325.

Single-head attention over N=4096 tokens, feature dim HW=4096:
  q, k, v = x[:,0], x[:,1], x[:,2] reshaped to [4096, 4096]
  out = softmax(0.5 * q @ k.T) @ v

Sharding: q rows split across 8 cores (M=512 rows each); k, v replicated.
Host pre-transposes q and k into PE-ready contraction-major layouts.

v2 design (vs the 85us/615us 2-pass baseline):
  * v and E in bf16 for phase 2 (same PE rate as f32r, half the v HBM
    traffic + SBUF). Scores stay f32r (precision-required).
  * j-blocks split into two halves H1/H2 with an exact flash-style
    combine, so the exp pass + phase 2 of H1 overlap phase 1 of H2 on
    the PE -- kills the serialization bubble that dropped the HAM clock
    to 1.2GHz in the baseline.
  * Per-half shift via the W-stat trick: W_h = sum_j exp(0.2*dp - 40)
    accumulated by ones^T @ exp matmuls during phase 1; shift_h =
    LSE_{0.2} >= rowmax_h, so exp(dp - shift_h) <= 1 (never overflows,
    deterministically). H2's exp uses shift_f = max(shift1, shift2), so
    only H1 needs rescaling: out = (alpha1 * O1 + O2) / (alpha1*l1 + l2).
  * O1 is parked in DRAM f32r (SBUF is full) and folded back into the
    H2 PSUM accumulation with one extra matmul per output tile using a
    diag(alpha1) stationary operand: po2 += diag(a1)^T @ O1.
  * Row stats (rowsums, alpha, 1/l) are produced in PE-broadcast layout
    and moved to per-partition layout with 8 tiny PE transposes.
  * kT slabs alternate between the sync and scalar HWDGE rings; v loads
    are 1MB [128,8,512] strided slabs.
"""
import sys

sys.path.insert(0, "/opt/trn_rl_repo")

import numpy as np
import ml_dtypes

import concourse.bass as bass
import concourse.tile as tile
from concourse import bacc, mybir
from concourse.bass_utils import run_bass_kernel_spmd

F32 = mybir.dt.float32
F32R = mybir.dt.float32r
BF16 = mybir.dt.bfloat16
EXP = mybir.ActivationFunctionType.Exp
LN = mybir.ActivationFunctionType.Ln
MULT = mybir.AluOpType.mult
ADD = mybir.AluOpType.add

N_CORES = 8
N = 4096          # tokens (keys)
D = 4096          # feature dim (H*W)
M = N // N_CORES  # q rows per core = 512
NJ = N // 128     # 32 key blocks
HJ = NJ // 2      # 16 key blocks per half
ND = D // 128     # 32 feature blocks
NI = M // 128     # 4 q-row blocks per core
NDT = D // 512    # 8 output column tiles
T_STAT = 0.2      # stage-1 temperature: exp(t*dp - 40) = exp(0.1*R - 40)
STAT_BIAS = 40.0


def _build_nc():
    nc = bacc.Bacc(None, target_bir_lowering=False, debug=False)

    # qT[p, db, i] = q[i, db*128+p]; kT[jb, p, db, jj] = k[jb*128+jj, db*128+p]
    qT_dram = nc.dram_tensor("qT", [128, ND, M], F32R, kind="ExternalInput")
    kT_dram = nc.dram_tensor("kT", [NJ, 128, ND, 128], F32R, kind="ExternalInput")
    v_dram = nc.dram_tensor("v", [N, D], BF16, kind="ExternalInput")
    id_dram = nc.dram_tensor("ident", [128, 128], F32, kind="ExternalInput")
    o_dram = nc.dram_tensor("o", [M, D], F32, kind="ExternalOutput")
    # H1 numerators parked here between the two pipeline stages
    o1_dram = nc.dram_tensor("o1scr", [M, D], F32R, kind="Internal")

    # v viewed [p, jb, d] so one dma_start grabs 8 j-blocks x 512 cols (1MB)
    v_r = v_dram.rearrange("(jb p) d -> p jb d", p=128)

    with tile.TileContext(nc) as tc:
        with (
            tc.tile_pool(name="persist", bufs=1) as persist,
            tc.tile_pool(name="s", bufs=16) as spool,
            tc.tile_pool(name="eT", bufs=33) as eTpool,
            tc.tile_pool(name="ett", bufs=2) as etpool,
            tc.tile_pool(name="o1st", bufs=2) as o1stpool,
            tc.tile_pool(name="tmp", bufs=2) as tmpool,
        ):
            ones_f = persist.tile([128, 128], F32, tag="ones_f")
            nc.vector.memset(ones_f[:], 1.0)
            ones_b = persist.tile([128, 128], BF16, tag="ones_b")
            nc.vector.tensor_copy(ones_b[:], ones_f[:])

            zero_b = persist.tile([128, 1], F32, tag="zero_b")
            nc.vector.memset(zero_b[:], 0.0)
            # stage-1 exp bias keeps W well under f32/exp breakdown range
            stat_b = persist.tile([128, 1], F32, tag="stat_b")
            nc.vector.memset(stat_b[:], -STAT_BIAS)

            ident_f = persist.tile([128, 128], F32, tag="ident_f")
            ident_r = persist.tile([128, 128], F32R, tag="ident_r")

            # broadcast-layout ([128, M], identical rows) row statistics
            sh2_1 = persist.tile([128, M], F32, tag="sh2_1")
            sh2_2 = persist.tile([128, M], F32, tag="sh2_2")
            sh2_f = persist.tile([128, M], F32, tag="sh2_f")
            w_ln = persist.tile([128, M], F32, tag="w_ln")
            a1_bc = persist.tile([128, M], F32, tag="a1_bc")
            rs1_bc = persist.tile([128, M], F32, tag="rs1_bc")
            ri_bc = persist.tile([128, M], F32, tag="ri_bc")
            # per-partition (transposed) stats
            a1col = persist.tile([128, NI], F32, tag="a1col")
            ricol = persist.tile([128, NI], F32, tag="ricol")
            diag_r = persist.tile([128, NI, 128], F32R, tag="diag_r")

            def exp_block(j, sh2):
                """eT(j) = exp(0.5*s(j) - shift) in bf16; frees s(j)."""
                t = tmpool.tile([128, M], F32, tag="tmp", name=f"tx{j}")
                nc.vector.tensor_sub(t[:], s_tiles[j][:], sh2[:])
                e = eTpool.tile([128, M], BF16, tag="eT", name=f"eT{j}")
                nc.scalar.activation(
                    out=e[:], in_=t[:], func=EXP, bias=zero_b[:], scale=0.5
                )
                eT_tiles[j] = e
                s_tiles[j] = None

            s_tiles = [None] * NJ
            eT_tiles = [None] * NJ

            with (
                tc.tile_pool(name="qT", bufs=1) as qTpool,
                tc.tile_pool(name="kT", bufs=3) as kTpool,
                tc.tile_pool(name="vtB", bufs=3) as vpoolB,
                tc.tile_pool(name="psS", bufs=2, space="PSUM") as psS,
                tc.tile_pool(name="psW", bufs=1, space="PSUM") as psWp,
                tc.tile_pool(name="psPO", bufs=4, space="PSUM") as psPO,
                tc.tile_pool(name="psR", bufs=1, space="PSUM") as psRp,
            ):
                v_pool_ref = [vpoolB]
                # kT(0) both halves on the sync ring so nothing queues
                # behind the 8.4MB qT load on scalar; j>=1 halves alternate.
                kT0A = kTpool.tile([128, ND // 2, 128], F32R, tag="kT",
                                   name="kT0A")
                nc.sync.dma_start(out=kT0A[:], in_=kT_dram[0, :, 0:ND // 2, :])
                kT0B = kTpool.tile([128, ND // 2, 128], F32R, tag="kT",
                                   name="kT0B")
                nc.sync.dma_start(out=kT0B[:], in_=kT_dram[0, :, ND // 2:, :])
                kT0 = (kT0A, kT0B)

                # qT in 8 chunks, split across both rings
                qT_parts = []
                for b in range(8):
                    qp = qTpool.tile([128, ND // 8, M], F32R, tag=f"qT{b}",
                                     name=f"qT{b}")
                    eng = nc.scalar if b < 6 else nc.sync
                    eng.dma_start(
                        out=qp[:],
                        in_=qT_dram[:, b * (ND // 8):(b + 1) * (ND // 8), :],
                    )
                    qT_parts.append(qp)
                nc.scalar.dma_start(out=ident_f[:], in_=id_dram[:, :])
                nc.vector.tensor_copy(ident_r[:], ident_f[:])

                def qT_slice(dblk):
                    return qT_parts[dblk // (ND // 8)][:, dblk % (ND // 8), :]

                psW_half = [None, None]

                def ph1_block(j, half):
                    """S^T(j) f32r matmuls + raw-score stash + W-stat."""
                    if j == 0:
                        kTA, kTB = kT0
                    else:
                        engA = nc.sync if (j % 2 == 0) else nc.scalar
                        engB = nc.scalar if (j % 2 == 0) else nc.sync
                        kTA = kTpool.tile([128, ND // 2, 128], F32R, tag="kT",
                                          name=f"kT{j}A")
                        engA.dma_start(out=kTA[:],
                                       in_=kT_dram[j, :, 0:ND // 2, :])
                        kTB = kTpool.tile([128, ND // 2, 128], F32R, tag="kT",
                                          name=f"kT{j}B")
                        engB.dma_start(out=kTB[:],
                                       in_=kT_dram[j, :, ND // 2:, :])
                    ps = psS.tile([128, M], F32, tag="S", name=f"ps{j}")
                    for dblk in range(ND):
                        kt = kTA if dblk < ND // 2 else kTB
                        nc.tensor.matmul(
                            ps[:],
                            kt[:, dblk % (ND // 2), :],
                            qT_slice(dblk),
                            start=(dblk == 0),
                            stop=(dblk == ND - 1),
                        )
                    st = spool.tile([128, M], F32, tag="s", name=f"s{j}")
                    nc.vector.tensor_copy(st[:], ps[:])
                    s_tiles[j] = st
                    ett = etpool.tile([128, M], BF16, tag="ett", name=f"et{j}")
                    nc.scalar.activation(
                        out=ett[:], in_=ps[:], func=EXP,
                        bias=stat_b[:], scale=0.5 * T_STAT,
                    )
                    if psW_half[half] is None:
                        psW_half[half] = psWp.tile([128, M], F32, tag="W",
                                                   name=f"psW{half}")
                    j0 = half * HJ
                    nc.tensor.matmul(
                        psW_half[half][:],
                        ones_b[:],
                        ett[:],
                        start=(j == j0),
                        stop=(j == j0 + HJ - 1),
                        skip_group_check=True,
                    )

                def sh2_from_W(half, sh2_out):
                    # 2*shift = (2/t)*(ln W + bias); psW rows are identical
                    nc.scalar.activation(
                        out=w_ln[:], in_=psW_half[half][:], func=LN,
                        bias=zero_b[:], scale=1.0,
                    )
                    nc.vector.tensor_scalar(
                        sh2_out[:], w_ln[:], 2.0 / T_STAT,
                        STAT_BIAS * 2.0 / T_STAT, MULT, ADD,
                    )

                v_tiles = {}

                def v_load(half, dt, sub):
                    """1MB slab: 8 j-blocks x 512 cols of v (bf16).

                    H1 slabs ride the gpsimd (SWDGE) queue so their
                    slot-pacing stalls never block the kT stream."""
                    vt = v_pool_ref[0].tile([128, 8, 512], BF16, tag="vt",
                                            name=f"v{half}_{dt}_{sub}")
                    jb0 = half * HJ + sub * 8
                    if half == 0:
                        eng = nc.gpsimd
                    else:
                        eng = nc.sync if (dt + sub) % 2 == 0 else nc.scalar
                    eng.dma_start(
                        out=vt[:],
                        in_=v_r[:, jb0:jb0 + 8, dt * 512:(dt + 1) * 512],
                    )
                    v_tiles[(half, dt, sub)] = vt

                def rs_mm(psR, j, half):
                    j0 = half * HJ
                    nc.tensor.matmul(
                        psR[:],
                        ones_b[:],
                        eT_tiles[j][:],
                        start=(j == j0),
                        stop=(j == j0 + HJ - 1),
                        skip_group_check=True,
                    )

                def ph2_chunk(half, dt, sub, po):
                    """8 j-blocks of the E@v accumulation for one dt tile."""
                    if (half, dt, sub) not in v_tiles:
                        v_load(half, dt, sub)
                    vt = v_tiles[(half, dt, sub)]
                    j0 = half * HJ
                    for jj in range(8):
                        j = j0 + sub * 8 + jj
                        for ib in range(NI):
                            nc.tensor.matmul(
                                po[ib][:],
                                eT_tiles[j][:, ib * 128:(ib + 1) * 128],
                                vt[:, jj, :],
                                start=(j == j0),
                                stop=(j == j0 + HJ - 1),
                                skip_group_check=True,
                            )
                    if sub == 1:
                        del v_tiles[(half, dt, sub)]
                        del v_tiles[(half, dt, 0)]

                # ---------------- A: phase 1 over H1 ----------------
                for j in range(HJ):
                    ph1_block(j, 0)
                # prefetch first phase-2 v slabs during A's tail
                v_load(0, 0, 0)
                v_load(0, 0, 1)

                sh2_from_W(0, sh2_1)

                # ---------------- B ----------------
                # exp + rowsum of H1 (ready as soon as sh2_1 lands)
                psR1 = psRp.tile([128, M], F32, tag="R", name="psR1")
                for j in range(HJ):
                    exp_block(j, sh2_1)
                    rs_mm(psR1, j, 0)
                nc.vector.tensor_copy(rs1_bc[:], psR1[:])

                def emit_chunk(t):
                    dt, sub = t // 2, t % 2
                    if sub == 0:
                        emit_chunk.po = [
                            psPO.tile([128, 512], F32, tag="po",
                                      name=f"po0_{dt}_{ib}")
                            for ib in range(NI)
                        ]
                    ph2_chunk(0, dt, sub, emit_chunk.po)
                    if sub == 1:
                        for ib in range(NI):
                            o1st = o1stpool.tile([128, 512], F32R, tag="o1st",
                                                 name=f"o1s{dt}_{ib}")
                            nc.vector.tensor_copy(o1st[:], emit_chunk.po[ib][:])
                            oeng = nc.sync if ib % 2 == 0 else nc.scalar
                            oeng.dma_start(
                                out=o1_dram[ib * 128:(ib + 1) * 128,
                                            dt * 512:(dt + 1) * 512],
                                in_=o1st[:],
                            )

                # phase-1(H2) at two blocks per round so its W stats close
                # ~halfway through B; phase-2(H1) chunks fill the PE gaps.
                for r in range(HJ // 2):
                    ph1_block(HJ + 2 * r, 1)
                    ph1_block(HJ + 2 * r + 1, 1)
                    if (0, r // 2, r % 2) not in v_tiles:
                        v_load(0, r // 2, r % 2)
                    emit_chunk(r)

                sh2_from_W(1, sh2_2)
                nc.vector.tensor_max(sh2_f[:], sh2_1[:], sh2_2[:])
                a1t = tmpool.tile([128, M], F32, tag="tmp", name="a1t")
                nc.vector.tensor_sub(a1t[:], sh2_1[:], sh2_f[:])
                nc.scalar.activation(
                    out=a1_bc[:], in_=a1t[:], func=EXP, bias=zero_b[:],
                    scale=0.5,
                )
                # exp H2 + its rowsums drain on ACT/PE alongside phase-2(H1)
                psR2 = psRp.tile([128, M], F32, tag="R", name="psR2")
                for j in range(HJ, NJ):
                    exp_block(j, sh2_f)
                    rs_mm(psR2, j, 1)
                # alpha1 to per-partition layout via the freed psW bank
                for ib in range(NI):
                    pt = psWp.tile([128, M], F32, tag="W", name=f"ta{ib}")
                    nc.tensor.transpose(
                        pt[:, 0:128], a1_bc[:, ib * 128:(ib + 1) * 128],
                        ident_f[:],
                    )
                    nc.vector.tensor_copy(a1col[:, ib:ib + 1], pt[:, 0:1])
                for ib in range(NI):
                    nc.vector.tensor_scalar_mul(
                        diag_r[:, ib, :], ident_r[:], a1col[:, ib:ib + 1]
                    )
                # l_f = a1*l1 + l2 ; 1/l_f ; transpose to per-partition
                # (w_ln is dead by now -- reuse it as the l_f scratch)
                nc.vector.tensor_mul(w_ln[:], a1_bc[:], rs1_bc[:])
                nc.vector.tensor_add(w_ln[:], w_ln[:], psR2[:])
                nc.vector.reciprocal(ri_bc[:], w_ln[:])
                for ib in range(NI):
                    pt = psWp.tile([128, M], F32, tag="W", name=f"tr{ib}")
                    nc.tensor.transpose(
                        pt[:, 0:128], ri_bc[:, ib * 128:(ib + 1) * 128],
                        ident_f[:],
                    )
                    nc.vector.tensor_copy(ricol[:, ib:ib + 1], pt[:, 0:1])

                # remaining phase-2(H1) chunks (PE tail of B)
                for t in range(HJ // 2, HJ):
                    if (0, t // 2, t % 2) not in v_tiles:
                        v_load(0, t // 2, t % 2)
                    emit_chunk(t)

            # phase 2 over H2 + merge of the rescaled H1 numerators;
            # psS/psW/psT and qT are closed: all 8 PSUM banks + SBUF room
            with (
                tc.tile_pool(name="psPO2", bufs=8, space="PSUM") as psPO2,
                tc.tile_pool(name="vtC", bufs=4) as vpoolC,
                tc.tile_pool(name="o1ld", bufs=4) as o1ldpool,
                tc.tile_pool(name="osb", bufs=4) as osbpool,
            ):
                v_pool_ref[0] = vpoolC
                for dt in range(NDT):
                    for sub in range(2):
                        if (1, dt, sub) not in v_tiles:
                            v_load(1, dt, sub)
                    po = [
                        psPO2.tile([128, 512], F32, tag="po2",
                                   name=f"po1_{dt}_{ib}")
                        for ib in range(NI)
                    ]
                    # merge first: po = diag(alpha1)^T @ O1, then += E2@v2.
                    # o1ld transfers are small and prefetch well, so the PE
                    # has work at each dt boundary while the v slab lands.
                    for ib in range(NI):
                        o1ld = o1ldpool.tile([128, 512], F32R, tag="o1ld",
                                             name=f"o1l{dt}_{ib}")
                        ieng = nc.sync if ib % 2 == 0 else nc.scalar
                        ieng.dma_start(
                            out=o1ld[:],
                            in_=o1_dram[ib * 128:(ib + 1) * 128,
                                        dt * 512:(dt + 1) * 512],
                        )
                        nc.tensor.matmul(
                            po[ib][:],
                            diag_r[:, ib, :],
                            o1ld[:],
                            start=True,
                            stop=False,
                            skip_group_check=True,
                        )
                    for sub in range(2):
                        vt = v_tiles[(1, dt, sub)]
                        j0 = HJ
                        for jj in range(8):
                            j = j0 + sub * 8 + jj
                            for ib in range(NI):
                                nc.tensor.matmul(
                                    po[ib][:],
                                    eT_tiles[j][:, ib * 128:(ib + 1) * 128],
                                    vt[:, jj, :],
                                    start=False,
                                    stop=(j == j0 + HJ - 1),
                                    skip_group_check=True,
                                )
                    del v_tiles[(1, dt, 0)]
                    del v_tiles[(1, dt, 1)]
                    # prefetch next dt's v slabs behind this dt's matmuls
                    if dt + 1 < NDT:
                        v_load(1, dt + 1, 0)
                        v_load(1, dt + 1, 1)
                    for ib in range(NI):
                        osb = osbpool.tile([128, 512], F32, tag="osb",
                                           name=f"ob{dt}_{ib}")
                        nc.vector.tensor_scalar_mul(
                            osb[:], po[ib][:], ricol[:, ib:ib + 1]
                        )
                        oeng = nc.scalar if ib % 2 == 0 else nc.sync
                        oeng.dma_start(
                            out=o_dram[ib * 128:(ib + 1) * 128,
                                       dt * 512:(dt + 1) * 512],
                            in_=osb[:],
                        )

    nc.compile()
    return nc


_NC_CACHE = None


def _get_nc():
    global _NC_CACHE
    if _NC_CACHE is None:
        _NC_CACHE = _build_nc()
    return _NC_CACHE


def _make_in_maps(x: np.ndarray) -> list:
    x = np.asarray(x)
    n, c, h, w = x.shape
    assert (n, c, h * w) == (N, 3, D), f"unexpected shape {x.shape}"
    xr = np.ascontiguousarray(x.reshape(n, c, h * w).transpose(1, 0, 2))
    q_full, k, v = xr[0], xr[1], xr[2]
    # kT[jb, p, db, jj] = k[jb*128+jj, db*128+p] -- per-(jb) contiguous 2MB
    kT = np.ascontiguousarray(
        k.reshape(NJ, 128, ND, 128).transpose(0, 3, 2, 1)
    )
    v_b = np.ascontiguousarray(v.astype(ml_dtypes.bfloat16))
    ident = np.eye(128, dtype=np.float32)
    in_maps = []
    for core in range(N_CORES):
        qc = q_full[core * M:(core + 1) * M]          # [M, D]
        # qT[p, db, i] = q[i, db*128+p]
        qTc = np.ascontiguousarray(
            qc.reshape(M, ND, 128).transpose(2, 1, 0)
        )
        in_maps.append({"qT": qTc, "kT": kT, "v": v_b, "ident": ident})
    return in_maps


def kernel(x: np.ndarray) -> np.ndarray:
    nc = _get_nc()
    res = run_bass_kernel_spmd(nc, _make_in_maps(x), core_ids=list(range(N_CORES)))
    out = np.concatenate([r["o"] for r in res.results], axis=0)
    return out.astype(np.float32)


# revision 34
# speedup vs baseline: 1.1409x; 1.1409x over previous
"""TRN2 Bass kernel for nn_Attention_86260123173325.

Single-head attention over N=4096 tokens, feature dim HW=4096:
  q, k, v = x[:,0], x[:,1], x[:,2] reshaped to [4096, 4096]
  out = softmax(0.5 * q @ k.T) @ v

Sharding: q rows split across 8 cores (M=512 rows each); k, v replicated.
Host pre-transposes q and k into PE-ready contraction-major layouts.

v2 design (vs the 85us/615us 2-pass baseline):
  * v and E in bf16 for phase 2 (same PE rate as f32r, half the v HBM
    traffic + SBUF). Scores stay f32r (precision-required).
  * j-blocks split into two halves H1/H2 with an exact flash-style
    combine, so the exp pass + phase 2 of H1 overlap phase 1 of H2 on
    the PE -- kills the serialization bubble that dropped the HAM clock
    to 1.2GHz in the baseline.
  * Per-half shift via the W-stat trick: W_h = sum_j exp(0.2*dp - 40)
    accumulated by ones^T @ exp matmuls during phase 1; shift_h =
    LSE_{0.2} >= rowmax_h, so exp(dp - shift_h) <= 1 (never overflows,
    deterministically). H2's exp uses shift_f = max(shift1, shift2), so
    only H1 needs rescaling: out = (alpha1 * O1 + O2) / (alpha1*l1 + l2).
  * O1 is parked in DRAM f32r (SBUF is full) and folded back into the
    H2 PSUM accumulation with one extra matmul per output tile using a
    diag(alpha1) stationary operand: po2 += diag(a1)^T @ O1.
  * Row stats (rowsums, alpha, 1/l) are produced in PE-broadcast layout
    and moved to per-partition layout with 8 tiny PE transposes.
  * kT slabs alternate between the sync and scalar HWDGE rings; v loads
    are 1MB [128,8,512] strided slabs.
"""
import sys

sys.path.insert(0, "/opt/trn_rl_repo")

import numpy as np
import ml_dtypes

import concourse.bass as bass
import concourse.tile as tile
from concourse import bacc, mybir
from concourse.bass_utils import run_bass_kernel_spmd

F32 = mybir.dt.float32
F32R = mybir.dt.float32r
BF16 = mybir.dt.bfloat16
EXP = mybir.ActivationFunctionType.Exp
LN = mybir.ActivationFunctionType.Ln
MULT = mybir.AluOpType.mult
ADD = mybir.AluOpType.add

N_CORES = 8
N = 4096          # tokens (keys)
D = 4096          # feature dim (H*W)
M = N // N_CORES  # q rows per core = 512
NJ = N // 128     # 32 key blocks
HJ = NJ // 2      # 16 key blocks per half
ND = D // 128     # 32 feature blocks
NI = M // 128     # 4 q-row blocks per core
NDT = D // 512    # 8 output column tiles
T_STAT = 0.2      # stage-1 temperature: exp(t*dp - 40) = exp(0.1*R - 40)
STAT_BIAS = 40.0


def _build_nc():
    nc = bacc.Bacc(None, target_bir_lowering=False, debug=False)

    # qT[p, db, i] = q[i, db*128+p]; kT[jb, p, db, jj] = k[jb*128+jj, db*128+p]
    qT_dram = nc.dram_tensor("qT", [128, ND, M], F32R, kind="ExternalInput")
    kT_dram = nc.dram_tensor("kT", [NJ, 128, ND, 128], F32R, kind="ExternalInput")
    v_dram = nc.dram_tensor("v", [N, D], BF16, kind="ExternalInput")
    id_dram = nc.dram_tensor("ident", [128, 128], F32, kind="ExternalInput")
    o_dram = nc.dram_tensor("o", [M, D], F32, kind="ExternalOutput")
    # H1 numerators parked here between the two pipeline stages
    o1_dram = nc.dram_tensor("o1scr", [M, D], F32R, kind="Internal")

    # v viewed [p, jb, d] so one dma_start grabs 8 j-blocks x 512 cols (1MB)
    v_r = v_dram.rearrange("(jb p) d -> p jb d", p=128)

    with tile.TileContext(nc) as tc:
        with (
            tc.tile_pool(name="persist", bufs=1) as persist,
            tc.tile_pool(name="s", bufs=16) as spool,
            tc.tile_pool(name="eT", bufs=17) as eTpool,
            tc.tile_pool(name="ett", bufs=2) as etpool,
            tc.tile_pool(name="o1st", bufs=2) as o1stpool,
            tc.tile_pool(name="tmp", bufs=2) as tmpool,
        ):
            ones_f = persist.tile([128, 128], F32, tag="ones_f")
            nc.vector.memset(ones_f[:], 1.0)
            ones_b = persist.tile([128, 128], BF16, tag="ones_b")
            nc.vector.tensor_copy(ones_b[:], ones_f[:])

            zero_b = persist.tile([128, 1], F32, tag="zero_b")
            nc.vector.memset(zero_b[:], 0.0)
            # stage-1 exp bias keeps W well under f32/exp breakdown range
            stat_b = persist.tile([128, 1], F32, tag="stat_b")
            nc.vector.memset(stat_b[:], -STAT_BIAS)

            ident_f = persist.tile([128, 128], F32, tag="ident_f")
            ident_r = persist.tile([128, 128], F32R, tag="ident_r")

            # broadcast-layout ([128, M], identical rows) row statistics
            sh2_1 = persist.tile([128, M], F32, tag="sh2_1")
            sh2_2 = persist.tile([128, M], F32, tag="sh2_2")
            sh2_f = persist.tile([128, M], F32, tag="sh2_f")
            w_ln = persist.tile([128, M], F32, tag="w_ln")
            a1_bc = persist.tile([128, M], F32, tag="a1_bc")
            rs1_bc = persist.tile([128, M], F32, tag="rs1_bc")
            ri_bc = persist.tile([128, M], F32, tag="ri_bc")
            # per-partition (transposed) stats
            a1col = persist.tile([128, NI], F32, tag="a1col")
            ricol = persist.tile([128, NI], F32, tag="ricol")
            diag_r = persist.tile([128, NI, 128], F32R, tag="diag_r")

            def exp_block(j, sh2):
                """eT(j) = exp(0.5*s(j) - shift) in bf16; frees s(j)."""
                t = tmpool.tile([128, M], F32, tag="tmp", name=f"tx{j}")
                nc.vector.tensor_sub(t[:], s_tiles[j][:], sh2[:])
                e = eTpool.tile([128, M], BF16, tag="eT", name=f"eT{j}")
                nc.scalar.activation(
                    out=e[:], in_=t[:], func=EXP, bias=zero_b[:], scale=0.5
                )
                eT_tiles[j] = e
                s_tiles[j] = None

            s_tiles = [None] * NJ
            eT_tiles = [None] * NJ

            with (
                tc.tile_pool(name="qT", bufs=1) as qTpool,
                tc.tile_pool(name="kT", bufs=4) as kTpool,
                tc.tile_pool(name="vtB", bufs=3) as vpoolB,
                tc.tile_pool(name="psS", bufs=2, space="PSUM") as psS,
                tc.tile_pool(name="psW", bufs=1, space="PSUM") as psWp,
                tc.tile_pool(name="psPO", bufs=4, space="PSUM") as psPO,
                tc.tile_pool(name="psR", bufs=1, space="PSUM") as psRp,
            ):
                v_pool_ref = [vpoolB]
                # kT(0) both halves on the sync ring so nothing queues
                # behind the 8.4MB qT load on scalar; j>=1 halves alternate.
                kT0A = kTpool.tile([128, ND // 2, 128], F32R, tag="kT",
                                   name="kT0A")
                nc.sync.dma_start(out=kT0A[:], in_=kT_dram[0, :, 0:ND // 2, :])
                kT0B = kTpool.tile([128, ND // 2, 128], F32R, tag="kT",
                                   name="kT0B")
                nc.sync.dma_start(out=kT0B[:], in_=kT_dram[0, :, ND // 2:, :])
                kT0 = (kT0A, kT0B)

                # qT in 8 chunks, split across both rings
                qT_parts = []
                for b in range(8):
                    qp = qTpool.tile([128, ND // 8, M], F32R, tag=f"qT{b}",
                                     name=f"qT{b}")
                    eng = nc.scalar if b < 6 else nc.sync
                    eng.dma_start(
                        out=qp[:],
                        in_=qT_dram[:, b * (ND // 8):(b + 1) * (ND // 8), :],
                    )
                    qT_parts.append(qp)
                nc.scalar.dma_start(out=ident_f[:], in_=id_dram[:, :])
                nc.vector.tensor_copy(ident_r[:], ident_f[:])

                def qT_slice(dblk):
                    return qT_parts[dblk // (ND // 8)][:, dblk % (ND // 8), :]

                psW_half = [None, None]

                def ph1_block(j, half):
                    """S^T(j) f32r matmuls + raw-score stash + W-stat."""
                    if j == 0:
                        kTA, kTB = kT0
                    else:
                        engA = nc.sync if (j % 2 == 0) else nc.scalar
                        engB = nc.scalar if (j % 2 == 0) else nc.sync
                        kTA = kTpool.tile([128, ND // 2, 128], F32R, tag="kT",
                                          name=f"kT{j}A")
                        engA.dma_start(out=kTA[:],
                                       in_=kT_dram[j, :, 0:ND // 2, :])
                        kTB = kTpool.tile([128, ND // 2, 128], F32R, tag="kT",
                                          name=f"kT{j}B")
                        engB.dma_start(out=kTB[:],
                                       in_=kT_dram[j, :, ND // 2:, :])
                    ps = psS.tile([128, M], F32, tag="S", name=f"ps{j}")
                    for dblk in range(ND):
                        kt = kTA if dblk < ND // 2 else kTB
                        nc.tensor.matmul(
                            ps[:],
                            kt[:, dblk % (ND // 2), :],
                            qT_slice(dblk),
                            start=(dblk == 0),
                            stop=(dblk == ND - 1),
                        )
                    st = spool.tile([128, M], F32, tag="s", name=f"s{j}")
                    nc.vector.tensor_copy(st[:], ps[:])
                    s_tiles[j] = st
                    ett = etpool.tile([128, M], BF16, tag="ett", name=f"et{j}")
                    nc.scalar.activation(
                        out=ett[:], in_=ps[:], func=EXP,
                        bias=stat_b[:], scale=0.5 * T_STAT,
                    )
                    if psW_half[half] is None:
                        psW_half[half] = psWp.tile([128, M], F32, tag="W",
                                                   name=f"psW{half}")
                    j0 = half * HJ
                    nc.tensor.matmul(
                        psW_half[half][:],
                        ones_b[:],
                        ett[:],
                        start=(j == j0),
                        stop=(j == j0 + HJ - 1),
                        skip_group_check=True,
                    )

                def sh2_from_W(half, sh2_out):
                    # 2*shift = (2/t)*(ln W + bias); psW rows are identical
                    nc.scalar.activation(
                        out=w_ln[:], in_=psW_half[half][:], func=LN,
                        bias=zero_b[:], scale=1.0,
                    )
                    nc.vector.tensor_scalar(
                        sh2_out[:], w_ln[:], 2.0 / T_STAT,
                        STAT_BIAS * 2.0 / T_STAT, MULT, ADD,
                    )

                v_tiles = {}

                def v_load(half, dt, sub):
                    """1MB slab: 8 j-blocks x 512 cols of v (bf16).

                    H1 slabs ride the gpsimd (SWDGE) queue so their
                    slot-pacing stalls never block the kT stream."""
                    vt = v_pool_ref[0].tile([128, 8, 512], BF16, tag="vt",
                                            name=f"v{half}_{dt}_{sub}")
                    jb0 = half * HJ + sub * 8
                    eng = nc.sync if (dt + sub) % 2 == 0 else nc.scalar
                    eng.dma_start(
                        out=vt[:],
                        in_=v_r[:, jb0:jb0 + 8, dt * 512:(dt + 1) * 512],
                    )
                    v_tiles[(half, dt, sub)] = vt

                def rs_mm(psR, j, half):
                    j0 = half * HJ
                    nc.tensor.matmul(
                        psR[:],
                        ones_b[:],
                        eT_tiles[j][:],
                        start=(j == j0),
                        stop=(j == j0 + HJ - 1),
                        skip_group_check=True,
                    )

                def ph2_chunk(half, dt, sub, po):
                    """8 j-blocks of the E@v accumulation for one dt tile."""
                    if (half, dt, sub) not in v_tiles:
                        v_load(half, dt, sub)
                    vt = v_tiles[(half, dt, sub)]
                    j0 = half * HJ
                    for jj in range(8):
                        j = j0 + sub * 8 + jj
                        for ib in range(NI):
                            nc.tensor.matmul(
                                po[ib][:],
                                eT_tiles[j][:, ib * 128:(ib + 1) * 128],
                                vt[:, jj, :],
                                start=(j == j0),
                                stop=(j == j0 + HJ - 1),
                                skip_group_check=True,
                            )
                    if sub == 1:
                        del v_tiles[(half, dt, sub)]
                        del v_tiles[(half, dt, 0)]

                # ---------------- A: phase 1 over H1 ----------------
                for j in range(HJ):
                    ph1_block(j, 0)
                # prefetch first phase-2 v slabs during A's tail
                v_load(0, 0, 0)
                v_load(0, 0, 1)

                sh2_from_W(0, sh2_1)

                # ---------------- B ----------------
                # exp + rowsum of H1 (ready as soon as sh2_1 lands)
                psR1 = psRp.tile([128, M], F32, tag="R", name="psR1")
                for j in range(HJ):
                    exp_block(j, sh2_1)
                    rs_mm(psR1, j, 0)
                nc.vector.tensor_copy(rs1_bc[:], psR1[:])

                def emit_chunk(t):
                    dt, sub = t // 2, t % 2
                    if sub == 0:
                        emit_chunk.po = [
                            psPO.tile([128, 512], F32, tag="po",
                                      name=f"po0_{dt}_{ib}")
                            for ib in range(NI)
                        ]
                    ph2_chunk(0, dt, sub, emit_chunk.po)
                    if sub == 1:
                        for ib in range(NI):
                            o1st = o1stpool.tile([128, 512], F32R, tag="o1st",
                                                 name=f"o1s{dt}_{ib}")
                            nc.vector.tensor_copy(o1st[:], emit_chunk.po[ib][:])
                            oeng = nc.sync if ib % 2 == 0 else nc.scalar
                            oeng.dma_start(
                                out=o1_dram[ib * 128:(ib + 1) * 128,
                                            dt * 512:(dt + 1) * 512],
                                in_=o1st[:],
                            )

                # interleave phase-1(H2) with phase-2(H1), one block : one
                # chunk per round
                for t in range(HJ):
                    ph1_block(HJ + t, 1)
                    if t + 2 < HJ and (0, (t + 2) // 2, (t + 2) % 2) not in v_tiles:
                        v_load(0, (t + 2) // 2, (t + 2) % 2)
                    emit_chunk(t)

                sh2_from_W(1, sh2_2)

            # ---------------- C ----------------
            nc.vector.tensor_max(sh2_f[:], sh2_1[:], sh2_2[:])
            a1t = tmpool.tile([128, M], F32, tag="tmp", name="a1t")
            nc.vector.tensor_sub(a1t[:], sh2_1[:], sh2_f[:])
            nc.scalar.activation(
                out=a1_bc[:], in_=a1t[:], func=EXP, bias=zero_b[:], scale=0.5
            )

            with (
                tc.tile_pool(name="psT", bufs=2, space="PSUM") as psT,
                tc.tile_pool(name="psR2", bufs=1, space="PSUM") as psR2p,
            ):
                # exp H2 with the final shift; start rowsums right away
                psR2 = psR2p.tile([128, M], F32, tag="R2", name="psR2")
                for j in range(HJ, NJ):
                    exp_block(j, sh2_f)
                    rs_mm(psR2, j, 1)
                    if j == HJ + 1:
                        # alpha1 transposes fill the PE while exps drain
                        for ib in range(NI):
                            pt = psT.tile([128, 128], F32, tag="T",
                                          name=f"ta{ib}")
                            nc.tensor.transpose(
                                pt[:], a1_bc[:, ib * 128:(ib + 1) * 128],
                                ident_f[:],
                            )
                            nc.vector.tensor_copy(
                                a1col[:, ib:ib + 1], pt[:, 0:1]
                            )
                        for ib in range(NI):
                            nc.vector.tensor_scalar_mul(
                                diag_r[:, ib, :], ident_r[:],
                                a1col[:, ib:ib + 1],
                            )
                # l_f = a1*l1 + l2 ; 1/l_f ; transpose to per-partition
                # (w_ln is dead by now -- reuse it as the l_f scratch)
                nc.vector.tensor_mul(w_ln[:], a1_bc[:], rs1_bc[:])
                nc.vector.tensor_add(w_ln[:], w_ln[:], psR2[:])
                nc.vector.reciprocal(ri_bc[:], w_ln[:])
                for ib in range(NI):
                    pt = psT.tile([128, 128], F32, tag="T", name=f"tr{ib}")
                    nc.tensor.transpose(
                        pt[:], ri_bc[:, ib * 128:(ib + 1) * 128], ident_f[:]
                    )
                    nc.vector.tensor_copy(ricol[:, ib:ib + 1], pt[:, 0:1])

            # phase 2 over H2 + merge of the rescaled H1 numerators;
            # psS/psW/psT and qT are closed: all 8 PSUM banks + SBUF room
            with (
                tc.tile_pool(name="psPO2", bufs=8, space="PSUM") as psPO2,
                tc.tile_pool(name="vtC", bufs=4) as vpoolC,
                tc.tile_pool(name="o1ld", bufs=4) as o1ldpool,
                tc.tile_pool(name="osb", bufs=4) as osbpool,
            ):
                v_pool_ref[0] = vpoolC
                for dt in range(NDT):
                    for sub in range(2):
                        if (1, dt, sub) not in v_tiles:
                            v_load(1, dt, sub)
                    po = [
                        psPO2.tile([128, 512], F32, tag="po2",
                                   name=f"po1_{dt}_{ib}")
                        for ib in range(NI)
                    ]
                    # merge first: po = diag(alpha1)^T @ O1, then += E2@v2.
                    # o1ld transfers are small and prefetch well, so the PE
                    # has work at each dt boundary while the v slab lands.
                    for ib in range(NI):
                        o1ld = o1ldpool.tile([128, 512], F32R, tag="o1ld",
                                             name=f"o1l{dt}_{ib}")
                        ieng = nc.sync if ib % 2 == 0 else nc.scalar
                        ieng.dma_start(
                            out=o1ld[:],
                            in_=o1_dram[ib * 128:(ib + 1) * 128,
                                        dt * 512:(dt + 1) * 512],
                        )
                        nc.tensor.matmul(
                            po[ib][:],
                            diag_r[:, ib, :],
                            o1ld[:],
                            start=True,
                            stop=False,
                            skip_group_check=True,
                        )
                    for sub in range(2):
                        vt = v_tiles[(1, dt, sub)]
                        j0 = HJ
                        for jj in range(8):
                            j = j0 + sub * 8 + jj
                            for ib in range(NI):
                                nc.tensor.matmul(
                                    po[ib][:],
                                    eT_tiles[j][:, ib * 128:(ib + 1) * 128],
                                    vt[:, jj, :],
                                    start=False,
                                    stop=(j == j0 + HJ - 1),
                                    skip_group_check=True,
                                )
                    del v_tiles[(1, dt, 0)]
                    del v_tiles[(1, dt, 1)]
                    # prefetch next dt's v slabs behind this dt's matmuls
                    if dt + 1 < NDT:
                        v_load(1, dt + 1, 0)
                        v_load(1, dt + 1, 1)
                    for ib in range(NI):
                        osb = osbpool.tile([128, 512], F32, tag="osb",
                                           name=f"ob{dt}_{ib}")
                        nc.vector.tensor_scalar_mul(
                            osb[:], po[ib][:], ricol[:, ib:ib + 1]
                        )
                        oeng = nc.scalar if ib % 2 == 0 else nc.sync
                        oeng.dma_start(
                            out=o_dram[ib * 128:(ib + 1) * 128,
                                       dt * 512:(dt + 1) * 512],
                            in_=osb[:],
                        )

    nc.compile()
    return nc


_NC_CACHE = None


def _get_nc():
    global _NC_CACHE
    if _NC_CACHE is None:
        _NC_CACHE = _build_nc()
    return _NC_CACHE


def _make_in_maps(x: np.ndarray) -> list:
    x = np.asarray(x)
    n, c, h, w = x.shape
    assert (n, c, h * w) == (N, 3, D), f"unexpected shape {x.shape}"
    xr = np.ascontiguousarray(x.reshape(n, c, h * w).transpose(1, 0, 2))
    q_full, k, v = xr[0], xr[1], xr[2]
    # kT[jb, p, db, jj] = k[jb*128+jj, db*128+p] -- per-(jb) contiguous 2MB
    kT = np.ascontiguousarray(
        k.reshape(NJ, 128, ND, 128).transpose(0, 3, 2, 1)
    )
    v_b = np.ascontiguousarray(v.astype(ml_dtypes.bfloat16))
    ident = np.eye(128, dtype=np.float32)
    in_maps = []
    for core in range(N_CORES):
        qc = q_full[core * M:(core + 1) * M]          # [M, D]
        # qT[p, db, i] = q[i, db*128+p]
        qTc = np.ascontiguousarray(
            qc.reshape(M, ND, 128).transpose(2, 1, 0)
        )
        in_maps.append({"qT": qTc, "kT": kT, "v": v_b, "ident": ident})
    return in_maps


def kernel(x: np.ndarray) -> np.ndarray:
    nc = _get_nc()
    res = run_bass_kernel_spmd(nc, _make_in_maps(x), core_ids=list(range(N_CORES)))
    out = np.concatenate([r["o"] for r in res.results], axis=0)
    return out.astype(np.float32)


# revision 39
# speedup vs baseline: 1.1455x; 1.0041x over previous
"""TRN2 Bass kernel for nn_Attention_86260123173325.

Single-head attention over N=4096 tokens, feature dim HW=4096:
  q, k, v = x[:,0], x[:,1], x[:,2] reshaped to [4096, 4096]
  out = softmax(0.5 * q @ k.T) @ v

Sharding: q rows split across 8 cores (M=512 rows each); k, v replicated.
Host pre-transposes q and k into PE-ready contraction-major layouts.

v2 design (vs the 85us/615us 2-pass baseline):
  * v and E in bf16 for phase 2 (same PE rate as f32r, half the v HBM
    traffic + SBUF). Scores stay f32r (precision-required).
  * j-blocks split into two halves H1/H2 with an exact flash-style
    combine, so the exp pass + phase 2 of H1 overlap phase 1 of H2 on
    the PE -- kills the serialization bubble that dropped the HAM clock
    to 1.2GHz in the baseline.
  * Per-half shift via the W-stat trick: W_h = sum_j exp(0.2*dp - 40)
    accumulated by ones^T @ exp matmuls during phase 1; shift_h =
    LSE_{0.2} >= rowmax_h, so exp(dp - shift_h) <= 1 (never overflows,
    deterministically). H2's exp uses shift_f = max(shift1, shift2), so
    only H1 needs rescaling: out = (alpha1 * O1 + O2) / (alpha1*l1 + l2).
  * O1 is parked in DRAM f32r (SBUF is full) and folded back into the
    H2 PSUM accumulation with one extra matmul per output tile using a
    diag(alpha1) stationary operand: po2 += diag(a1)^T @ O1.
  * Row stats (rowsums, alpha, 1/l) are produced in PE-broadcast layout
    and moved to per-partition layout with 8 tiny PE transposes.
  * kT slabs alternate between the sync and scalar HWDGE rings; v loads
    are 1MB [128,8,512] strided slabs.
"""
import sys

sys.path.insert(0, "/opt/trn_rl_repo")

import numpy as np
import ml_dtypes

import concourse.bass as bass
import concourse.tile as tile
from concourse import bacc, mybir
from concourse.bass_utils import run_bass_kernel_spmd

F32 = mybir.dt.float32
F32R = mybir.dt.float32r
BF16 = mybir.dt.bfloat16
EXP = mybir.ActivationFunctionType.Exp
LN = mybir.ActivationFunctionType.Ln
MULT = mybir.AluOpType.mult
ADD = mybir.AluOpType.add

N_CORES = 8
N = 4096          # tokens (keys)
D = 4096          # feature dim (H*W)
M = N // N_CORES  # q rows per core = 512
NJ = N // 128     # 32 key blocks
HJ = NJ // 2      # 16 key blocks per half
ND = D // 128     # 32 feature blocks
NI = M // 128     # 4 q-row blocks per core
NDT = D // 512    # 8 output column tiles
T_STAT = 0.2      # stage-1 temperature: exp(t*dp - 40) = exp(0.1*R - 40)
STAT_BIAS = 40.0


def _build_nc():
    nc = bacc.Bacc(None, target_bir_lowering=False, debug=False)

    # qT[p, db, i] = q[i, db*128+p]; kT[jb, p, db, jj] = k[jb*128+jj, db*128+p]
    qT_dram = nc.dram_tensor("qT", [128, ND, M], F32R, kind="ExternalInput")
    kT_dram = nc.dram_tensor("kT", [NJ, 128, ND, 128], F32R, kind="ExternalInput")
    v_dram = nc.dram_tensor("v", [N, D], BF16, kind="ExternalInput")
    id_dram = nc.dram_tensor("ident", [128, 128], F32, kind="ExternalInput")
    o_dram = nc.dram_tensor("o", [M, D], F32, kind="ExternalOutput")
    # H1 numerators parked here between the two pipeline stages
    o1_dram = nc.dram_tensor("o1scr", [M, D], F32R, kind="Internal")

    # v viewed [p, jb, d] so one dma_start grabs 8 j-blocks x 512 cols (1MB)
    v_r = v_dram.rearrange("(jb p) d -> p jb d", p=128)

    with tile.TileContext(nc) as tc:
        with (
            tc.tile_pool(name="persist", bufs=1) as persist,
            tc.tile_pool(name="s", bufs=16) as spool,
            tc.tile_pool(name="eT", bufs=17) as eTpool,
            tc.tile_pool(name="ett", bufs=2) as etpool,
            tc.tile_pool(name="o1st", bufs=2) as o1stpool,
            tc.tile_pool(name="tmp", bufs=2) as tmpool,
        ):
            ones_f = persist.tile([128, 128], F32, tag="ones_f")
            nc.vector.memset(ones_f[:], 1.0)
            ones_b = persist.tile([128, 128], BF16, tag="ones_b")
            nc.vector.tensor_copy(ones_b[:], ones_f[:])

            zero_b = persist.tile([128, 1], F32, tag="zero_b")
            nc.vector.memset(zero_b[:], 0.0)
            # stage-1 exp bias keeps W well under f32/exp breakdown range
            stat_b = persist.tile([128, 1], F32, tag="stat_b")
            nc.vector.memset(stat_b[:], -STAT_BIAS)

            ident_f = persist.tile([128, 128], F32, tag="ident_f")
            ident_r = persist.tile([128, 128], F32R, tag="ident_r")

            # broadcast-layout ([128, M], identical rows) row statistics
            sh2_1 = persist.tile([128, M], F32, tag="sh2_1")
            sh2_2 = persist.tile([128, M], F32, tag="sh2_2")
            sh2_f = persist.tile([128, M], F32, tag="sh2_f")
            w_ln = persist.tile([128, M], F32, tag="w_ln")
            a1_bc = persist.tile([128, M], F32, tag="a1_bc")
            rs1_bc = persist.tile([128, M], F32, tag="rs1_bc")
            ri_bc = persist.tile([128, M], F32, tag="ri_bc")
            # per-partition (transposed) stats
            a1col = persist.tile([128, NI], F32, tag="a1col")
            ricol = persist.tile([128, NI], F32, tag="ricol")
            diag_r = persist.tile([128, NI, 128], F32R, tag="diag_r")

            def exp_block(j, sh2):
                """eT(j) = exp(0.5*s(j) - shift) in bf16; frees s(j)."""
                t = tmpool.tile([128, M], F32, tag="tmp", name=f"tx{j}")
                nc.vector.tensor_sub(t[:], s_tiles[j][:], sh2[:])
                e = eTpool.tile([128, M], BF16, tag="eT", name=f"eT{j}")
                nc.scalar.activation(
                    out=e[:], in_=t[:], func=EXP, bias=zero_b[:], scale=0.5
                )
                eT_tiles[j] = e
                s_tiles[j] = None

            s_tiles = [None] * NJ
            eT_tiles = [None] * NJ

            with (
                tc.tile_pool(name="qT", bufs=1) as qTpool,
                tc.tile_pool(name="kT", bufs=4) as kTpool,
                tc.tile_pool(name="vtB", bufs=3) as vpoolB,
                tc.tile_pool(name="psS", bufs=2, space="PSUM") as psS,
                tc.tile_pool(name="psW", bufs=1, space="PSUM") as psWp,
                tc.tile_pool(name="psPO", bufs=4, space="PSUM") as psPO,
                tc.tile_pool(name="psR", bufs=1, space="PSUM") as psRp,
            ):
                v_pool_ref = [vpoolB]
                # kT(0) both halves on the sync ring so nothing queues
                # behind the 8.4MB qT load on scalar; j>=1 halves alternate.
                kT0A = kTpool.tile([128, ND // 2, 128], F32R, tag="kT",
                                   name="kT0A")
                nc.sync.dma_start(out=kT0A[:], in_=kT_dram[0, :, 0:ND // 2, :])
                kT0B = kTpool.tile([128, ND // 2, 128], F32R, tag="kT",
                                   name="kT0B")
                nc.sync.dma_start(out=kT0B[:], in_=kT_dram[0, :, ND // 2:, :])
                kT0 = (kT0A, kT0B)

                # qT in 8 chunks, split across both rings
                qT_parts = []
                for b in range(8):
                    qp = qTpool.tile([128, ND // 8, M], F32R, tag=f"qT{b}",
                                     name=f"qT{b}")
                    eng = nc.scalar if b < 6 else nc.sync
                    eng.dma_start(
                        out=qp[:],
                        in_=qT_dram[:, b * (ND // 8):(b + 1) * (ND // 8), :],
                    )
                    qT_parts.append(qp)
                nc.scalar.dma_start(out=ident_f[:], in_=id_dram[:, :])
                nc.vector.tensor_copy(ident_r[:], ident_f[:])

                def qT_slice(dblk):
                    return qT_parts[dblk // (ND // 8)][:, dblk % (ND // 8), :]

                psW_half = [None, None]

                def ph1_block(j, half):
                    """S^T(j) f32r matmuls + raw-score stash + W-stat."""
                    if j == 0:
                        kTA, kTB = kT0
                    else:
                        if j <= 2:
                            # early blocks: keep clear of the qT queue
                            engA = engB = nc.sync
                        else:
                            engA = nc.sync if (j % 2 == 0) else nc.scalar
                            engB = nc.scalar if (j % 2 == 0) else nc.sync
                        kTA = kTpool.tile([128, ND // 2, 128], F32R, tag="kT",
                                          name=f"kT{j}A")
                        engA.dma_start(out=kTA[:],
                                       in_=kT_dram[j, :, 0:ND // 2, :])
                        kTB = kTpool.tile([128, ND // 2, 128], F32R, tag="kT",
                                          name=f"kT{j}B")
                        engB.dma_start(out=kTB[:],
                                       in_=kT_dram[j, :, ND // 2:, :])
                    ps = psS.tile([128, M], F32, tag="S", name=f"ps{j}")
                    for dblk in range(ND):
                        kt = kTA if dblk < ND // 2 else kTB
                        nc.tensor.matmul(
                            ps[:],
                            kt[:, dblk % (ND // 2), :],
                            qT_slice(dblk),
                            start=(dblk == 0),
                            stop=(dblk == ND - 1),
                        )
                    st = spool.tile([128, M], F32, tag="s", name=f"s{j}")
                    nc.vector.tensor_copy(st[:], ps[:])
                    s_tiles[j] = st
                    ett = etpool.tile([128, M], BF16, tag="ett", name=f"et{j}")
                    nc.scalar.activation(
                        out=ett[:], in_=ps[:], func=EXP,
                        bias=stat_b[:], scale=0.5 * T_STAT,
                    )
                    if psW_half[half] is None:
                        psW_half[half] = psWp.tile([128, M], F32, tag="W",
                                                   name=f"psW{half}")
                    j0 = half * HJ
                    nc.tensor.matmul(
                        psW_half[half][:],
                        ones_b[:],
                        ett[:],
                        start=(j == j0),
                        stop=(j == j0 + HJ - 1),
                        skip_group_check=True,
                    )

                def sh2_from_W(half, sh2_out):
                    # 2*shift = (2/t)*(ln W + bias); psW rows are identical
                    nc.scalar.activation(
                        out=w_ln[:], in_=psW_half[half][:], func=LN,
                        bias=zero_b[:], scale=1.0,
                    )
                    nc.vector.tensor_scalar(
                        sh2_out[:], w_ln[:], 2.0 / T_STAT,
                        STAT_BIAS * 2.0 / T_STAT, MULT, ADD,
                    )

                v_tiles = {}

                def v_load(half, dt, sub):
                    """1MB slab: 8 j-blocks x 512 cols of v (bf16).

                    H1 slabs ride the gpsimd (SWDGE) queue so their
                    slot-pacing stalls never block the kT stream."""
                    vt = v_pool_ref[0].tile([128, 8, 512], BF16, tag="vt",
                                            name=f"v{half}_{dt}_{sub}")
                    jb0 = half * HJ + sub * 8
                    eng = nc.sync if (dt + sub) % 2 == 0 else nc.scalar
                    eng.dma_start(
                        out=vt[:],
                        in_=v_r[:, jb0:jb0 + 8, dt * 512:(dt + 1) * 512],
                    )
                    v_tiles[(half, dt, sub)] = vt

                def rs_mm(psR, j, half):
                    j0 = half * HJ
                    nc.tensor.matmul(
                        psR[:],
                        ones_b[:],
                        eT_tiles[j][:],
                        start=(j == j0),
                        stop=(j == j0 + HJ - 1),
                        skip_group_check=True,
                    )

                def ph2_chunk(half, dt, sub, po):
                    """8 j-blocks of the E@v accumulation for one dt tile."""
                    if (half, dt, sub) not in v_tiles:
                        v_load(half, dt, sub)
                    vt = v_tiles[(half, dt, sub)]
                    j0 = half * HJ
                    for jj in range(8):
                        j = j0 + sub * 8 + jj
                        for ib in range(NI):
                            nc.tensor.matmul(
                                po[ib][:],
                                eT_tiles[j][:, ib * 128:(ib + 1) * 128],
                                vt[:, jj, :],
                                start=(j == j0),
                                stop=(j == j0 + HJ - 1),
                                skip_group_check=True,
                            )
                    if sub == 1:
                        del v_tiles[(half, dt, sub)]
                        del v_tiles[(half, dt, 0)]

                # ---------------- A: phase 1 over H1 ----------------
                for j in range(HJ):
                    ph1_block(j, 0)
                # prefetch first phase-2 v slabs during A's tail
                v_load(0, 0, 0)
                v_load(0, 0, 1)

                sh2_from_W(0, sh2_1)

                # ---------------- B ----------------
                # exp + rowsum of H1 (ready as soon as sh2_1 lands)
                psR1 = psRp.tile([128, M], F32, tag="R", name="psR1")
                for j in range(HJ):
                    exp_block(j, sh2_1)
                    rs_mm(psR1, j, 0)
                nc.vector.tensor_copy(rs1_bc[:], psR1[:])

                def emit_chunk(t):
                    dt, sub = t // 2, t % 2
                    if sub == 0:
                        emit_chunk.po = [
                            psPO.tile([128, 512], F32, tag="po",
                                      name=f"po0_{dt}_{ib}")
                            for ib in range(NI)
                        ]
                    ph2_chunk(0, dt, sub, emit_chunk.po)
                    if sub == 1:
                        for ib in range(NI):
                            o1st = o1stpool.tile([128, 512], F32R, tag="o1st",
                                                 name=f"o1s{dt}_{ib}")
                            nc.vector.tensor_copy(o1st[:], emit_chunk.po[ib][:])
                            oeng = nc.sync if ib % 2 == 0 else nc.scalar
                            oeng.dma_start(
                                out=o1_dram[ib * 128:(ib + 1) * 128,
                                            dt * 512:(dt + 1) * 512],
                                in_=o1st[:],
                            )

                # interleave phase-1(H2) with phase-2(H1), one block : one
                # chunk per round
                for t in range(HJ):
                    ph1_block(HJ + t, 1)
                    if t + 2 < HJ and (0, (t + 2) // 2, (t + 2) % 2) not in v_tiles:
                        v_load(0, (t + 2) // 2, (t + 2) % 2)
                    emit_chunk(t)

                sh2_from_W(1, sh2_2)

            # ---------------- C ----------------
            nc.vector.tensor_max(sh2_f[:], sh2_1[:], sh2_2[:])
            a1t = tmpool.tile([128, M], F32, tag="tmp", name="a1t")
            nc.vector.tensor_sub(a1t[:], sh2_1[:], sh2_f[:])
            nc.scalar.activation(
                out=a1_bc[:], in_=a1t[:], func=EXP, bias=zero_b[:], scale=0.5
            )

            ctx_vtC = tc.tile_pool(name="vtC", bufs=4)
            ctx_o1ld = tc.tile_pool(name="o1ld", bufs=4)
            vpoolC = ctx_vtC.__enter__()
            o1ldpool = ctx_o1ld.__enter__()
            with (
                tc.tile_pool(name="psT", bufs=2, space="PSUM") as psT,
                tc.tile_pool(name="psR2", bufs=1, space="PSUM") as psR2p,
            ):
                # prefetch phase-2(H2)'s first operands during the exp drain
                v_pool_ref[0] = vpoolC
                v_load(1, 0, 0)
                v_load(1, 0, 1)
                o1_pre = {}
                for ib in range(NI):
                    o1ld = o1ldpool.tile([128, 512], F32R, tag="o1ld",
                                         name=f"o1l0_{ib}")
                    ieng = nc.sync if ib % 2 == 0 else nc.scalar
                    ieng.dma_start(
                        out=o1ld[:],
                        in_=o1_dram[ib * 128:(ib + 1) * 128, 0:512],
                    )
                    o1_pre[(0, ib)] = o1ld
                # exp H2 with the final shift; start rowsums right away
                psR2 = psR2p.tile([128, M], F32, tag="R2", name="psR2")
                for j in range(HJ, NJ):
                    exp_block(j, sh2_f)
                    rs_mm(psR2, j, 1)
                    if j == HJ + 1:
                        # alpha1 transposes fill the PE while exps drain
                        for ib in range(NI):
                            pt = psT.tile([128, 128], F32, tag="T",
                                          name=f"ta{ib}")
                            nc.tensor.transpose(
                                pt[:], a1_bc[:, ib * 128:(ib + 1) * 128],
                                ident_f[:],
                            )
                            nc.vector.tensor_copy(
                                a1col[:, ib:ib + 1], pt[:, 0:1]
                            )
                        for ib in range(NI):
                            nc.vector.tensor_scalar_mul(
                                diag_r[:, ib, :], ident_r[:],
                                a1col[:, ib:ib + 1],
                            )
                # l_f = a1*l1 + l2 ; 1/l_f ; transpose to per-partition
                # (w_ln is dead by now -- reuse it as the l_f scratch)
                nc.vector.tensor_mul(w_ln[:], a1_bc[:], rs1_bc[:])
                nc.vector.tensor_add(w_ln[:], w_ln[:], psR2[:])
                nc.vector.reciprocal(ri_bc[:], w_ln[:])
                for ib in range(NI):
                    pt = psT.tile([128, 128], F32, tag="T", name=f"tr{ib}")
                    nc.tensor.transpose(
                        pt[:], ri_bc[:, ib * 128:(ib + 1) * 128], ident_f[:]
                    )
                    nc.vector.tensor_copy(ricol[:, ib:ib + 1], pt[:, 0:1])

            # phase 2 over H2 + merge of the rescaled H1 numerators;
            # psS/psW/psT and qT are closed: all 8 PSUM banks + SBUF room
            with (
                tc.tile_pool(name="psPO2", bufs=8, space="PSUM") as psPO2,
                tc.tile_pool(name="osb", bufs=4) as osbpool,
            ):
                for dt in range(NDT):
                    for sub in range(2):
                        if (1, dt, sub) not in v_tiles:
                            v_load(1, dt, sub)
                    po = [
                        psPO2.tile([128, 512], F32, tag="po2",
                                   name=f"po1_{dt}_{ib}")
                        for ib in range(NI)
                    ]
                    # merge first: po = diag(alpha1)^T @ O1, then += E2@v2.
                    # o1ld transfers are small and prefetch well, so the PE
                    # has work at each dt boundary while the v slab lands.
                    for ib in range(NI):
                        o1ld = o1_pre.pop((dt, ib), None)
                        if o1ld is None:
                            o1ld = o1ldpool.tile([128, 512], F32R,
                                                 tag="o1ld",
                                                 name=f"o1l{dt}_{ib}")
                            ieng = nc.sync if ib % 2 == 0 else nc.scalar
                            ieng.dma_start(
                                out=o1ld[:],
                                in_=o1_dram[ib * 128:(ib + 1) * 128,
                                            dt * 512:(dt + 1) * 512],
                            )
                        nc.tensor.matmul(
                            po[ib][:],
                            diag_r[:, ib, :],
                            o1ld[:],
                            start=True,
                            stop=False,
                            skip_group_check=True,
                        )
                    for sub in range(2):
                        vt = v_tiles[(1, dt, sub)]
                        j0 = HJ
                        for jj in range(8):
                            j = j0 + sub * 8 + jj
                            for ib in range(NI):
                                nc.tensor.matmul(
                                    po[ib][:],
                                    eT_tiles[j][:, ib * 128:(ib + 1) * 128],
                                    vt[:, jj, :],
                                    start=False,
                                    stop=(j == j0 + HJ - 1),
                                    skip_group_check=True,
                                )
                    del v_tiles[(1, dt, 0)]
                    del v_tiles[(1, dt, 1)]
                    # prefetch next dt's v slabs behind this dt's matmuls
                    if dt + 1 < NDT:
                        v_load(1, dt + 1, 0)
                        v_load(1, dt + 1, 1)
                    for ib in range(NI):
                        osb = osbpool.tile([128, 512], F32, tag="osb",
                                           name=f"ob{dt}_{ib}")
                        nc.vector.tensor_scalar_mul(
                            osb[:], po[ib][:], ricol[:, ib:ib + 1]
                        )
                        oeng = nc.scalar if ib % 2 == 0 else nc.sync
                        oeng.dma_start(
                            out=o_dram[ib * 128:(ib + 1) * 128,
                                       dt * 512:(dt + 1) * 512],
                            in_=osb[:],
                        )
            ctx_o1ld.__exit__(None, None, None)
            ctx_vtC.__exit__(None, None, None)

    nc.compile()
    return nc


_NC_CACHE = None


def _get_nc():
    global _NC_CACHE
    if _NC_CACHE is None:
        _NC_CACHE = _build_nc()
    return _NC_CACHE


def _make_in_maps(x: np.ndarray) -> list:
    x = np.asarray(x)
    n, c, h, w = x.shape
    assert (n, c, h * w) == (N, 3, D), f"unexpected shape {x.shape}"
    xr = np.ascontiguousarray(x.reshape(n, c, h * w).transpose(1, 0, 2))
    q_full, k, v = xr[0], xr[1], xr[2]
    # kT[jb, p, db, jj] = k[jb*128+jj, db*128+p] -- per-(jb) contiguous 2MB
    kT = np.ascontiguousarray(
        k.reshape(NJ, 128, ND, 128).transpose(0, 3, 2, 1)
    )
    v_b = np.ascontiguousarray(v.astype(ml_dtypes.bfloat16))
    ident = np.eye(128, dtype=np.float32)
    in_maps = []
    for core in range(N_CORES):
        qc = q_full[core * M:(core + 1) * M]          # [M, D]
        # qT[p, db, i] = q[i, db*128+p]
        qTc = np.ascontiguousarray(
            qc.reshape(M, ND, 128).transpose(2, 1, 0)
        )
        in_maps.append({"qT": qTc, "kT": kT, "v": v_b, "ident": ident})
    return in_maps


def kernel(x: np.ndarray) -> np.ndarray:
    nc = _get_nc()
    res = run_bass_kernel_spmd(nc, _make_in_maps(x), core_ids=list(range(N_CORES)))
    out = np.concatenate([r["o"] for r in res.results], axis=0)
    return out.astype(np.float32)
